# revision 33
# baseline (speedup 1.0000x reference)
"""AlphaModel (relation-gated message passing) Trainium2 kernel.

Strategy (pure data parallel, per sharding hint):
  - Shard the 8M edges across 8 NeuronCores (1M each, zero-padded to a tile
    multiple).
  - Host precomputes g = concat(M.reshape(64,9), beta)[rels]  (tiny-table row
    gather, 12 f32 per edge) and streams it; there is no per-element LUT
    primitive on TRN2 that beats streaming (PE is 1 col/cycle, Pool gather
    ucode ~14 cyc/edge, indirect DMA ~0.34ns/descriptor).
  - Device does everything else: 3x3 matvec, sparsemax (via the simplex
    projection identity tau = max(mx-1, (sm-mn-1)/2, (sm-1)/3)), entropy,
    cosine similarity, scaling - in fp32 planar layout with custom fused DVE
    ops plus ACT for Ln/Sqrt/Square.

Output: alpha [8M, 3] float32.
"""

import sys

if "/opt/trn_rl_repo" not in sys.path:
    sys.path.insert(0, "/opt/trn_rl_repo")

import numpy as np

import concourse.bacc as bacc
import concourse.mybir as mybir
from concourse.bass_utils import run_bass_kernel_spmd
from concourse.tile import TileContext

N_CORES = 8
PDIM = 128

AF = mybir.ActivationFunctionType
OP = mybir.AluOpType
F32 = mybir.dt.float32

# --------------------------------------------------------------------------
# Custom fused DVE ops (registered once per process; compiled into the NEFF's
# per-kernel DVE table - documented extension point, no firmware change).
# --------------------------------------------------------------------------
_OPS_CACHE: dict = {}


def _custom_ops():
    if _OPS_CACHE:
        return _OPS_CACHE
    from concourse import dve_ops
    from concourse.dve_ops import DveOp, OPS, _SUB_OPCODE_FOR_NAME
    from concourse.dve_spec import (
        C0,
        C1,
        One,
        Spec,
        Src0,
        Src1,
        _has_src1,
        lower,
        maxx,
        relu,
    )
    from concourse.dve_uop import DveOpSpec

    existing = {op.name: op for op in OPS}

    def mk(key, name, body):
        if name in existing:
            _OPS_CACHE[key] = existing[name]
            return
        if name not in _SUB_OPCODE_FOR_NAME:
            row = max(_SUB_OPCODE_FOR_NAME.values()) + 1
            assert row < 0x20, "custom DVE opcode rows exhausted"
            _SUB_OPCODE_FOR_NAME[name] = row
        spec = Spec(body=body)
        shas = {}
        for ver in ("v3", "v4"):
            uops = lower(spec, ver=ver)
            s = DveOpSpec(
                name=name,
                opcode=_SUB_OPCODE_FOR_NAME[name],
                uops=uops,
                rd1_en=_has_src1(spec),
            )
            shas[ver] = s.sha(ver)
        op = DveOp(name, spec, subdim=False, uops_sha=shas)
        OPS.append(op)
        dve_ops.CUSTOM_DVE_SPECS[name] = spec
        _OPS_CACHE[key] = op

    # tau candidates: max((sm - mn - 1)*0.5, (sm - 1)/3)
    mk("tau_a", "ANT_TAU_A", maxx((Src0 - Src1 - One) * C0, (Src0 - One) * C1))
    # tau = max(mx - 1, d)
    mk("tau_b", "ANT_TAU_B", maxx(Src0 - One, Src1))
    # sparsemax threshold: relu(x - tau)
    mk("relusub", "ANT_RELUSUB", relu(Src0 - Src1))
    # z = max(a + b, eps)
    mk("addmax", "ANT_ADDMAX", maxx(Src0 + Src1, C0))
    # cos = a*b + 0.1
    mk("fmac", "ANT_FMA_C", Src0 * Src1 + C0)
    # scale = (a*21)*b
    mk("smul", "ANT_SMUL", (Src0 * C0) * Src1)
    # out = max(a*b, 0.001)
    mk("maxmul", "ANT_MAXMUL", maxx(Src0 * Src1, C0))
    return _OPS_CACHE


# --------------------------------------------------------------------------
# Bass program
# --------------------------------------------------------------------------
_PROG_CACHE: dict = {}


def _build_program(z_eps: float, scale_factor: float, B: int, T: int):
    """One SPMD program; every core runs the same code on its own shard."""
    ops = _custom_ops()
    # Bacc (not raw Bass): its compile() runs generate_event_semaphores,
    # which legalizes multi-event-sem waits the DVE/CTRL structs can't carry.
    nc = bacc.Bacc(
        "TRN2",
        target_bir_lowering=False,
        num_devices=N_CORES,
        dynamic_dma_scratch_size=8192,
    )
    e_pad = PDIM * B * T

    # Single fused input stream per tile (one DMA -> one DMA-sem wait per
    # consumer; the DVE TT struct only tolerates a single event-sem wait).
    # Per tile, per partition: [3B child | 3B parent | 12B gathered-table].
    xin_d = nc.dram_tensor("xin", [e_pad * 18], F32, kind="ExternalInput")
    out_d = nc.dram_tensor("alpha", [e_pad * 3], F32, kind="ExternalOutput")

    xin_v = xin_d[:].rearrange("(t p c) -> t p c", t=T, p=PDIM)
    out_v = out_d[:].rearrange("(t p c) -> t p c", t=T, p=PDIM)

    with TileContext(nc) as tc:
        with (
            tc.tile_pool(name="io", bufs=2) as iop,
            tc.tile_pool(name="scr", bufs=1) as sp,
        ):
            for t in range(T):
                xin = iop.tile([PDIM, 18 * B], F32, tag="xin", name=f"xin{t}")
                ot = iop.tile([PDIM, 3 * B], F32, tag="ot", name=f"ot{t}")
                nc.sync.dma_start(xin[:], xin_v[t])

                # Planar layout: 18 contiguous planes of B per partition:
                # [cp0 cp1 cp2 | pp0 pp1 pp2 | m00..m22 | b0 b1 b2]
                cp_all = xin[:, 0 : 3 * B]
                pp_all = xin[:, 3 * B : 6 * B]
                mrow = [xin[:, (6 + 3 * i) * B : (9 + 3 * i) * B] for i in range(3)]
                b_all = xin[:, 15 * B : 18 * B]

                # output stays planar (3 planes of B); host re-interleaves

                def pl(tag):
                    return sp.tile(
                        [PDIM, B], F32, tag=tag, name=f"{tag}{t}", bufs=2
                    )[:]

                V = nc.vector
                A = nc.scalar

                def cdve(op_key, out, in0, in1, s0=0.0, s1=0.0, imm2=0.0):
                    V._custom_dve(
                        ops[op_key], out=out, in0=in0, in1=in1, s0=s0, s1=s1, imm2=imm2
                    )

                # ww: 6 planes; first half doubles as w1, second as l_all.
                # Keeping them in ONE tile lets paired sum-chains run as
                # single strided-wide adds into separate 2-plane outputs.
                ww = sp.tile([PDIM, 6 * B], F32, tag="ww", name=f"ww{t}")[:]
                w1 = ww[:, 0 : 3 * B]
                l_all = ww[:, 3 * B : 6 * B]
                c_all = sp.tile([PDIM, 3 * B], F32, tag="call", name=f"call{t}")[:]
                p_all = sp.tile([PDIM, 3 * B], F32, tag="pall", name=f"pall{t}")[:]

                def bcast3(plane_ap):
                    """[P,B] plane broadcast to [P,3,B] via a 0-stride dim."""
                    return plane_ap.rearrange("p (u b) -> p u b", u=1).broadcast_to(
                        [PDIM, 3, B]
                    )

                def pair_sum3(src6, out2, dma_tail=False):
                    """out2[:, k*B+e] = sum_j src6[:, (3k+j)*B+e] for k=0,1.
                    dma_tail: final accumulate runs on the (idle) DMA engines
                    via SWDGE cce-add instead of the saturated DVE."""
                    v = src6.rearrange("p (n b) -> p n b", b=B)
                    x = v[:, 0:6:3, :]
                    y = v[:, 1:6:3, :]
                    z = v[:, 2:6:3, :]
                    o = out2.rearrange("p (n b) -> p n b", b=B)
                    V.tensor_add(o, x, y)
                    if dma_tail:
                        nc.gpsimd.dma_start(o, z, accum_op=OP.add)
                    else:
                        V.tensor_add(o, o, z)

                # ---- c_raw = M[rel] @ child: rows 0,1 in one 6-plane mul
                # (cp broadcast across the two rows), row 2 separately.
                cp2 = cp_all.rearrange("p (u x) -> p u x", u=1).broadcast_to(
                    [PDIM, 2, 3 * B]
                )
                V.tensor_mul(
                    ww.rearrange("p (u x) -> p u x", x=3 * B),
                    xin[:, 6 * B : 12 * B].rearrange("p (u x) -> p u x", x=3 * B),
                    cp2,
                )
                pair_sum3(ww, c_all[:, 0 : 2 * B])  # c0|c1
                V.tensor_mul(w1, mrow[2], cp_all)
                c2 = c_all[:, 2 * B : 3 * B]
                V.tensor_add(c2, w1[:, 0:B], w1[:, B : 2 * B])
                V.tensor_add(c2, c2, w1[:, 2 * B : 3 * B])

                # ---- sparsemax (simplex projection, d=3):
                # tau = max(mx-1, (sm-mn-1)/2, (sm-1)/3); out = relu(x - tau)
                def sparsemax(x_all, out_all, tag):
                    x = [x_all[:, j * B : (j + 1) * B] for j in range(3)]
                    mx, mn, sm, dd = (
                        pl(f"mx{tag}"),
                        pl(f"mn{tag}"),
                        pl(f"sm{tag}"),
                        pl(f"dd{tag}"),
                    )
                    V.tensor_max(mx, x[0], x[1])
                    V.tensor_max(mx, mx, x[2])
                    V.tensor_tensor(mn, x[0], x[1], OP.min)
                    V.tensor_tensor(mn, mn, x[2], OP.min)
                    V.tensor_add(sm, x[0], x[1])
                    nc.gpsimd.dma_start(sm, x[2], accum_op=OP.add)
                    cdve("tau_a", dd, sm, mn, s0=0.5, s1=1.0 / 3.0)
                    cdve("tau_b", dd, mx, dd)
                    ov = out_all.rearrange("p (n b) -> p n b", b=B)
                    xv = x_all.rearrange("p (n b) -> p n b", b=B)
                    cdve("relusub", ov, xv, bcast3(dd))
                    return mx, mn, sm, dd

                # c = sparsemax(c_raw); the reference's second application is
                # an exact no-op (projection idempotence) up to ~1e-7: skipped.
                lzs = sparsemax(c_all, c_all, "c")[0]  # mx plane recycled
                ncs = sparsemax(pp_all, p_all, "p")[0]  # mx plane recycled

                # ---- z = max(p + c, eps); entropy = ln(zs) - sum(z ln z)/zs
                # zq = [zs | szl] computed as one paired sum over ww
                w1v = w1.rearrange("p (n b) -> p n b", b=B)
                cvv = c_all.rearrange("p (n b) -> p n b", b=B)
                pvv = p_all.rearrange("p (n b) -> p n b", b=B)
                cdve("addmax", w1v, cvv, pvv, s0=z_eps)  # w1 <- z (wide)
                A.activation(l_all, w1, AF.Ln)  # wide ln
                V.tensor_mul(l_all, w1, l_all)  # l <- z*ln z (wide)
                zq = sp.tile([PDIM, 2 * B], F32, tag="zq", name=f"zq{t}", bufs=2)[:]
                pair_sum3(ww, zq, dma_tail=True)  # zq <- [zs | szl]
                zs = zq[:, 0:B]
                szl = zq[:, B : 2 * B]
                A.activation(lzs, zs, AF.Ln)
                A.activation(zs, lzs, AF.Exp, scale=-1.0)  # zs <- 1/zs
                V.tensor_mul(szl, szl, zs)
                V.tensor_sub(lzs, lzs, szl)  # lzs <- entropy

                # ---- cos = 0.1 + (p.c) / sqrt((p.p)*(c.c))
                V.tensor_mul(w1, p_all, c_all)  # w1 <- pc (wide)
                A.square(l_all, p_all)  # wide
                nq = sp.tile([PDIM, 2 * B], F32, tag="nq", name=f"nq{t}", bufs=2)[:]
                pair_sum3(ww, nq, dma_tail=True)  # nq <- [p.c | p.p]
                num = nq[:, 0:B]
                nps = nq[:, B : 2 * B]
                A.square(l_all, c_all)  # wide
                V.tensor_add(ncs, l_all[:, 0:B], l_all[:, B : 2 * B])
                nc.gpsimd.dma_start(ncs, l_all[:, 2 * B : 3 * B], accum_op=OP.add)
                # 1/(|p||c|) = exp(-(ln nps + ln ncs)/2)
                A.activation(nps, nps, AF.Ln)
                A.activation(ncs, ncs, AF.Ln)
                V.tensor_add(ncs, nps, ncs)
                A.activation(ncs, ncs, AF.Exp, scale=-0.5)
                cdve("fmac", num, num, ncs, s0=0.1)  # num <- cos

                # ---- alpha0 = p + b*(c-p) (wide, into w1)
                V.tensor_sub(w1, c_all, p_all)
                V.tensor_mul(w1, w1, b_all)
                V.tensor_add(w1, p_all, w1)

                # ---- scale = sf*cos/entropy; out = max(alpha0*scale, 1e-3)
                # 1/entropy = exp(-ln entropy)
                A.activation(lzs, lzs, AF.Ln)
                A.activation(lzs, lzs, AF.Exp, scale=-1.0)
                cdve("smul", num, num, lzs, s0=scale_factor)  # num <- scale
                cdve("maxmul", ot[:].rearrange("p (n b) -> p n b", b=B), w1v,
                     bcast3(num), s0=0.001)

                nc.sync.dma_start(out_v[t], ot[:])

    nc.compile()
    return nc


def _get_program(z_eps: float, scale_factor: float, B: int, T: int):
    key = (round(z_eps, 9), round(scale_factor, 9), B, T)
    if key not in _PROG_CACHE:
        _PROG_CACHE[key] = _build_program(z_eps, scale_factor, B, T)
    return _PROG_CACHE[key]


# --------------------------------------------------------------------------
# Host entry point
# --------------------------------------------------------------------------
_B = 656
_T = 12
E_PAD = PDIM * _B * _T  # 1,007,616 >= 1,000,000


def _fused_shard(
    child: np.ndarray,
    prnt: np.ndarray,
    g: np.ndarray,
    k: int,
    e: int,
    e_pad: int,
    B: int,
    T: int,
) -> np.ndarray:
    """Per-core fused input stream [T, 128, 18, B]: per tile & partition, 18
    contiguous planes of B: [cp0 cp1 cp2 | pp0 pp1 pp2 | m00..m22 | b0 b1 b2],
    flattened f32 (planar keeps every device access contiguous)."""

    def pad(a):
        sl = a[k * e : (k + 1) * e]
        out = np.zeros((e_pad, a.shape[1]), dtype=np.float32)
        out[: sl.shape[0]] = sl
        # [e_pad, w] -> [T, 128, B, w] -> planar [T, 128, w, B]
        return out.reshape(T, PDIM, B, -1).transpose(0, 1, 3, 2)

    return np.ascontiguousarray(
        np.concatenate([pad(child), pad(prnt), pad(g)], axis=2)
    ).reshape(-1)


def _run(inputs: dict, trace: bool = False):
    child = np.asarray(inputs["child_probs"], dtype=np.float32)
    prnt = np.asarray(inputs["prnt_probs"], dtype=np.float32)
    M = np.asarray(inputs["M"], dtype=np.float32)
    beta = np.asarray(inputs["beta"], dtype=np.float32)
    rels = np.asarray(inputs["rels"])
    z_eps = float(np.asarray(inputs["z_epsilon"]))
    sf = float(np.asarray(inputs["scale_factor"]))

    n = rels.shape[0]
    assert n % N_CORES == 0
    e = n // N_CORES
    assert e <= E_PAD

    t12 = np.concatenate([M.reshape(M.shape[0], 9), beta], axis=1).astype(np.float32)
    g = t12[rels]  # [N, 12]

    nc = _get_program(z_eps, sf, _B, _T)
    in_maps = [
        {"xin": _fused_shard(child, prnt, g, k, e, E_PAD, _B, _T)}
        for k in range(N_CORES)
    ]
    res = run_bass_kernel_spmd(nc, in_maps, core_ids=list(range(N_CORES)), trace=trace)
    # device emits planar [T, 128, 3, B]; re-interleave to [E, 3]
    outs = [
        r["alpha"]
        .reshape(_T, PDIM, 3, _B)
        .transpose(0, 1, 3, 2)
        .reshape(E_PAD, 3)[:e]
        for r in res.results
    ]
    return np.concatenate(outs, axis=0), res


def kernel(**inputs) -> np.ndarray:
    out, _ = _run(inputs)
    return out


def kernel_traced(**inputs):
    """Returns (output, BassKernelResults-with-profile) for test harnesses."""
    return _run(inputs, trace=True)


# revision 38
# speedup vs baseline: 1.3134x; 1.3134x over previous
"""AlphaModel (relation-gated message passing) Trainium2 kernel.

Strategy (pure data parallel, per sharding hint):
  - Shard the 8M edges across 8 NeuronCores (1M each, zero-padded to a tile
    multiple).
  - Host precomputes g = concat(M.reshape(64,9), beta)[rels]  (tiny-table row
    gather, 12 f32 per edge) and streams it; there is no per-element LUT
    primitive on TRN2 that beats streaming (PE is 1 col/cycle, Pool gather
    ucode ~14 cyc/edge, indirect DMA ~0.34ns/descriptor).
  - Device does everything else: 3x3 matvec, sparsemax (via the simplex
    projection identity tau = max(mx-1, (sm-mn-1)/2, (sm-1)/3)), entropy,
    cosine similarity, scaling - in fp32 planar layout with custom fused DVE
    ops plus ACT for Ln/Sqrt/Square.

Output: alpha [8M, 3] float32.
"""

import sys

if "/opt/trn_rl_repo" not in sys.path:
    sys.path.insert(0, "/opt/trn_rl_repo")

import numpy as np

import concourse.bacc as bacc
import concourse.mybir as mybir
from concourse.bass_utils import run_bass_kernel_spmd
from concourse.tile import TileContext

N_CORES = 8
PDIM = 128

AF = mybir.ActivationFunctionType
OP = mybir.AluOpType
F32 = mybir.dt.float32

# --------------------------------------------------------------------------
# Custom fused DVE ops (registered once per process; compiled into the NEFF's
# per-kernel DVE table - documented extension point, no firmware change).
# --------------------------------------------------------------------------
_OPS_CACHE: dict = {}


def _custom_ops():
    if _OPS_CACHE:
        return _OPS_CACHE
    from concourse import dve_ops
    from concourse.dve_ops import DveOp, OPS, _SUB_OPCODE_FOR_NAME
    from concourse.dve_spec import (
        C0,
        C1,
        One,
        Spec,
        Src0,
        Src1,
        _has_src1,
        lower,
        maxx,
        relu,
    )
    from concourse.dve_uop import DveOpSpec

    existing = {op.name: op for op in OPS}

    def mk(key, name, body):
        if name in existing:
            _OPS_CACHE[key] = existing[name]
            return
        if name not in _SUB_OPCODE_FOR_NAME:
            row = max(_SUB_OPCODE_FOR_NAME.values()) + 1
            assert row < 0x20, "custom DVE opcode rows exhausted"
            _SUB_OPCODE_FOR_NAME[name] = row
        spec = Spec(body=body)
        shas = {}
        for ver in ("v3", "v4"):
            uops = lower(spec, ver=ver)
            s = DveOpSpec(
                name=name,
                opcode=_SUB_OPCODE_FOR_NAME[name],
                uops=uops,
                rd1_en=_has_src1(spec),
            )
            shas[ver] = s.sha(ver)
        op = DveOp(name, spec, subdim=False, uops_sha=shas)
        OPS.append(op)
        dve_ops.CUSTOM_DVE_SPECS[name] = spec
        _OPS_CACHE[key] = op

    # tau candidates: max((sm - mn - 1)*0.5, (sm - 1)/3)
    mk("tau_a", "ANT_TAU_A", maxx((Src0 - Src1 - One) * C0, (Src0 - One) * C1))
    # tau = max(mx - 1, d)
    mk("tau_b", "ANT_TAU_B", maxx(Src0 - One, Src1))
    # sparsemax threshold: relu(x - tau)
    mk("relusub", "ANT_RELUSUB", relu(Src0 - Src1))
    # z = max(a + b, eps)
    mk("addmax", "ANT_ADDMAX", maxx(Src0 + Src1, C0))
    # cos = a*b + 0.1
    mk("fmac", "ANT_FMA_C", Src0 * Src1 + C0)
    # scale = (a*21)*b
    mk("smul", "ANT_SMUL", (Src0 * C0) * Src1)
    # out = max(a*b, 0.001)
    mk("maxmul", "ANT_MAXMUL", maxx(Src0 * Src1, C0))
    return _OPS_CACHE


# --------------------------------------------------------------------------
# Bass program
# --------------------------------------------------------------------------
_PROG_CACHE: dict = {}


def _build_program(z_eps: float, scale_factor: float, B: int, T: int):
    """One SPMD program; every core runs the same code on its own shard."""
    ops = _custom_ops()
    # Bacc (not raw Bass): its compile() runs generate_event_semaphores,
    # which legalizes multi-event-sem waits the DVE/CTRL structs can't carry.
    nc = bacc.Bacc(
        "TRN2",
        target_bir_lowering=False,
        num_devices=N_CORES,
        dynamic_dma_scratch_size=8192,
    )
    e_pad = PDIM * B * T

    # Single fused input stream per tile (one DMA -> one DMA-sem wait per
    # consumer; the DVE TT struct only tolerates a single event-sem wait).
    # Per tile, per partition: [3B child | 3B parent | 12B gathered-table].
    xin_d = nc.dram_tensor("xin", [e_pad * 18], F32, kind="ExternalInput")
    out_d = nc.dram_tensor("alpha", [e_pad * 3], F32, kind="ExternalOutput")

    xin_v = xin_d[:].rearrange("(t p c) -> t p c", t=T, p=PDIM)
    out_v = out_d[:].rearrange("(t p c) -> t p c", t=T, p=PDIM)

    with TileContext(nc) as tc:
        with (
            tc.tile_pool(name="io", bufs=2) as iop,
            tc.tile_pool(name="scr", bufs=1) as sp,
        ):
            for t in range(T):
                xin = iop.tile([PDIM, 18 * B], F32, tag="xin", name=f"xin{t}")
                ot = iop.tile([PDIM, 3 * B], F32, tag="ot", name=f"ot{t}")
                nc.sync.dma_start(xin[:], xin_v[t])

                # Planar layout: 18 contiguous planes of B per partition:
                # [cp0 cp1 cp2 | pp0 pp1 pp2 | m00..m22 | b0 b1 b2]
                cp_all = xin[:, 0 : 3 * B]
                pp_all = xin[:, 3 * B : 6 * B]
                mrow = [xin[:, (6 + 3 * i) * B : (9 + 3 * i) * B] for i in range(3)]
                b_all = xin[:, 15 * B : 18 * B]

                # output stays planar (3 planes of B); host re-interleaves

                def pl(tag):
                    return sp.tile(
                        [PDIM, B], F32, tag=tag, name=f"{tag}{t}", bufs=2
                    )[:]

                V = nc.vector
                A = nc.scalar

                def cdve(op_key, out, in0, in1, s0=0.0, s1=0.0, imm2=0.0):
                    V._custom_dve(
                        ops[op_key], out=out, in0=in0, in1=in1, s0=s0, s1=s1, imm2=imm2
                    )

                # ww: 6 planes; first half doubles as w1, second as l_all.
                # Keeping them in ONE tile lets paired sum-chains run as
                # single strided-wide adds into separate 2-plane outputs.
                ww = sp.tile([PDIM, 6 * B], F32, tag="ww", name=f"ww{t}")[:]
                w1 = ww[:, 0 : 3 * B]
                l_all = ww[:, 3 * B : 6 * B]
                c_all = sp.tile([PDIM, 3 * B], F32, tag="call", name=f"call{t}")[:]
                p_all = sp.tile([PDIM, 3 * B], F32, tag="pall", name=f"pall{t}")[:]

                def bcast3(plane_ap):
                    """[P,B] plane broadcast to [P,3,B] via a 0-stride dim."""
                    return plane_ap.rearrange("p (u b) -> p u b", u=1).broadcast_to(
                        [PDIM, 3, B]
                    )

                def pair_sum3(src6, out2):
                    """out2[:, k*B+e] = sum_j src6[:, (3k+j)*B+e] for k=0,1."""
                    v = src6.rearrange("p (n b) -> p n b", b=B)
                    x = v[:, 0:6:3, :]
                    y = v[:, 1:6:3, :]
                    z = v[:, 2:6:3, :]
                    o = out2.rearrange("p (n b) -> p n b", b=B)
                    V.tensor_add(o, x, y)
                    V.tensor_add(o, o, z)

                # ---- c_raw = M[rel] @ child: rows 0,1 in one 6-plane mul
                # (cp broadcast across the two rows), row 2 separately.
                cp2 = cp_all.rearrange("p (u x) -> p u x", u=1).broadcast_to(
                    [PDIM, 2, 3 * B]
                )
                V.tensor_mul(
                    ww.rearrange("p (u x) -> p u x", x=3 * B),
                    xin[:, 6 * B : 12 * B].rearrange("p (u x) -> p u x", x=3 * B),
                    cp2,
                )
                pair_sum3(ww, c_all[:, 0 : 2 * B])  # c0|c1
                V.tensor_mul(w1, mrow[2], cp_all)
                c2 = c_all[:, 2 * B : 3 * B]
                V.tensor_add(c2, w1[:, 0:B], w1[:, B : 2 * B])
                V.tensor_add(c2, c2, w1[:, 2 * B : 3 * B])

                # ---- sparsemax (simplex projection, d=3):
                # tau = max(mx-1, (sm-mn-1)/2, (sm-1)/3); out = relu(x - tau)
                def sparsemax(x_all, out_all, tag):
                    x = [x_all[:, j * B : (j + 1) * B] for j in range(3)]
                    mx, mn, sm, dd = (
                        pl(f"mx{tag}"),
                        pl(f"mn{tag}"),
                        pl(f"sm{tag}"),
                        pl(f"dd{tag}"),
                    )
                    V.tensor_max(mx, x[0], x[1])
                    V.tensor_max(mx, mx, x[2])
                    V.tensor_tensor(mn, x[0], x[1], OP.min)
                    V.tensor_tensor(mn, mn, x[2], OP.min)
                    V.tensor_add(sm, x[0], x[1])
                    V.tensor_add(sm, sm, x[2])
                    cdve("tau_a", dd, sm, mn, s0=0.5, s1=1.0 / 3.0)
                    cdve("tau_b", dd, mx, dd)
                    ov = out_all.rearrange("p (n b) -> p n b", b=B)
                    xv = x_all.rearrange("p (n b) -> p n b", b=B)
                    cdve("relusub", ov, xv, bcast3(dd))
                    return mx, mn, sm, dd

                # c = sparsemax(c_raw); the reference's second application is
                # an exact no-op (projection idempotence) up to ~1e-7: skipped.
                lzs = sparsemax(c_all, c_all, "c")[0]  # mx plane recycled
                ncs = sparsemax(pp_all, p_all, "p")[0]  # mx plane recycled

                # ---- z = max(p + c, eps); entropy = ln(zs) - sum(z ln z)/zs
                # zq = [zs | szl] computed as one paired sum over ww
                w1v = w1.rearrange("p (n b) -> p n b", b=B)
                cvv = c_all.rearrange("p (n b) -> p n b", b=B)
                pvv = p_all.rearrange("p (n b) -> p n b", b=B)
                cdve("addmax", w1v, cvv, pvv, s0=z_eps)  # w1 <- z (wide)
                A.activation(l_all, w1, AF.Ln)  # wide ln
                V.tensor_mul(l_all, w1, l_all)  # l <- z*ln z (wide)
                zq = sp.tile([PDIM, 2 * B], F32, tag="zq", name=f"zq{t}", bufs=2)[:]
                pair_sum3(ww, zq)  # zq <- [zs | szl]
                zs = zq[:, 0:B]
                szl = zq[:, B : 2 * B]
                A.activation(lzs, zs, AF.Ln)
                A.activation(zs, lzs, AF.Exp, scale=-1.0)  # zs <- 1/zs
                V.tensor_mul(szl, szl, zs)
                V.tensor_sub(lzs, lzs, szl)  # lzs <- entropy

                # ---- cos = 0.1 + (p.c) / sqrt((p.p)*(c.c))
                V.tensor_mul(w1, p_all, c_all)  # w1 <- pc (wide)
                A.square(l_all, p_all)  # wide
                nq = sp.tile([PDIM, 2 * B], F32, tag="nq", name=f"nq{t}", bufs=2)[:]
                pair_sum3(ww, nq)  # nq <- [p.c | p.p]
                num = nq[:, 0:B]
                nps = nq[:, B : 2 * B]
                A.square(l_all, c_all)  # wide
                V.tensor_add(ncs, l_all[:, 0:B], l_all[:, B : 2 * B])
                V.tensor_add(ncs, ncs, l_all[:, 2 * B : 3 * B])
                # 1/(|p||c|) = exp(-(ln nps + ln ncs)/2)
                A.activation(nps, nps, AF.Ln)
                A.activation(ncs, ncs, AF.Ln)
                V.tensor_add(ncs, nps, ncs)
                A.activation(ncs, ncs, AF.Exp, scale=-0.5)
                cdve("fmac", num, num, ncs, s0=0.1)  # num <- cos

                # ---- alpha0 = p + b*(c-p) (wide, into w1)
                V.tensor_sub(w1, c_all, p_all)
                V.tensor_mul(w1, w1, b_all)
                V.tensor_add(w1, p_all, w1)

                # ---- scale = sf*cos/entropy; out = max(alpha0*scale, 1e-3)
                # 1/entropy = exp(-ln entropy)
                A.activation(lzs, lzs, AF.Ln)
                A.activation(lzs, lzs, AF.Exp, scale=-1.0)
                cdve("smul", num, num, lzs, s0=scale_factor)  # num <- scale
                cdve("maxmul", ot[:].rearrange("p (n b) -> p n b", b=B), w1v,
                     bcast3(num), s0=0.001)

                nc.sync.dma_start(out_v[t], ot[:])

    nc.compile()
    return nc


def _get_program(z_eps: float, scale_factor: float, B: int, T: int):
    key = (round(z_eps, 9), round(scale_factor, 9), B, T)
    if key not in _PROG_CACHE:
        _PROG_CACHE[key] = _build_program(z_eps, scale_factor, B, T)
    return _PROG_CACHE[key]


# --------------------------------------------------------------------------
# Host entry point
# --------------------------------------------------------------------------
_B = 656
_T = 12
E_PAD = PDIM * _B * _T  # 1,007,616 >= 1,000,000


def _fused_shard(
    child: np.ndarray,
    prnt: np.ndarray,
    g: np.ndarray,
    k: int,
    e: int,
    e_pad: int,
    B: int,
    T: int,
) -> np.ndarray:
    """Per-core fused input stream [T, 128, 18, B]: per tile & partition, 18
    contiguous planes of B: [cp0 cp1 cp2 | pp0 pp1 pp2 | m00..m22 | b0 b1 b2],
    flattened f32 (planar keeps every device access contiguous)."""

    def pad(a):
        sl = a[k * e : (k + 1) * e]
        out = np.zeros((e_pad, a.shape[1]), dtype=np.float32)
        out[: sl.shape[0]] = sl
        # [e_pad, w] -> [T, 128, B, w] -> planar [T, 128, w, B]
        return out.reshape(T, PDIM, B, -1).transpose(0, 1, 3, 2)

    return np.ascontiguousarray(
        np.concatenate([pad(child), pad(prnt), pad(g)], axis=2)
    ).reshape(-1)


def _run(inputs: dict, trace: bool = False):
    child = np.asarray(inputs["child_probs"], dtype=np.float32)
    prnt = np.asarray(inputs["prnt_probs"], dtype=np.float32)
    M = np.asarray(inputs["M"], dtype=np.float32)
    beta = np.asarray(inputs["beta"], dtype=np.float32)
    rels = np.asarray(inputs["rels"])
    z_eps = float(np.asarray(inputs["z_epsilon"]))
    sf = float(np.asarray(inputs["scale_factor"]))

    n = rels.shape[0]
    assert n % N_CORES == 0
    e = n // N_CORES
    assert e <= E_PAD

    t12 = np.concatenate([M.reshape(M.shape[0], 9), beta], axis=1).astype(np.float32)
    g = t12[rels]  # [N, 12]

    nc = _get_program(z_eps, sf, _B, _T)
    in_maps = [
        {"xin": _fused_shard(child, prnt, g, k, e, E_PAD, _B, _T)}
        for k in range(N_CORES)
    ]
    res = run_bass_kernel_spmd(nc, in_maps, core_ids=list(range(N_CORES)), trace=trace)
    # device emits planar [T, 128, 3, B]; re-interleave to [E, 3]
    outs = [
        r["alpha"]
        .reshape(_T, PDIM, 3, _B)
        .transpose(0, 1, 3, 2)
        .reshape(E_PAD, 3)[:e]
        for r in res.results
    ]
    return np.concatenate(outs, axis=0), res


def kernel(**inputs) -> np.ndarray:
    out, _ = _run(inputs)
    return out


def kernel_traced(**inputs):
    """Returns (output, BassKernelResults-with-profile) for test harnesses."""
    return _run(inputs, trace=True)


# revision 40
# speedup vs baseline: 1.3335x; 1.0153x over previous
"""AlphaModel (relation-gated message passing) Trainium2 kernel.

Strategy (pure data parallel, per sharding hint):
  - Shard the 8M edges across 8 NeuronCores (1M each, zero-padded to a tile
    multiple).
  - Host precomputes g = concat(M.reshape(64,9), beta)[rels]  (tiny-table row
    gather, 12 f32 per edge) and streams it; there is no per-element LUT
    primitive on TRN2 that beats streaming (PE is 1 col/cycle, Pool gather
    ucode ~14 cyc/edge, indirect DMA ~0.34ns/descriptor).
  - Device does everything else: 3x3 matvec, sparsemax (via the simplex
    projection identity tau = max(mx-1, (sm-mn-1)/2, (sm-1)/3)), entropy,
    cosine similarity, scaling - in fp32 planar layout with custom fused DVE
    ops plus ACT for Ln/Sqrt/Square.

Output: alpha [8M, 3] float32.
"""

import sys

if "/opt/trn_rl_repo" not in sys.path:
    sys.path.insert(0, "/opt/trn_rl_repo")

import numpy as np

import concourse.bacc as bacc
import concourse.mybir as mybir
from concourse.bass_utils import run_bass_kernel_spmd
from concourse.tile import TileContext

N_CORES = 8
PDIM = 128

AF = mybir.ActivationFunctionType
OP = mybir.AluOpType
F32 = mybir.dt.float32

# --------------------------------------------------------------------------
# Custom fused DVE ops (registered once per process; compiled into the NEFF's
# per-kernel DVE table - documented extension point, no firmware change).
# --------------------------------------------------------------------------
_OPS_CACHE: dict = {}


def _custom_ops():
    if _OPS_CACHE:
        return _OPS_CACHE
    from concourse import dve_ops
    from concourse.dve_ops import DveOp, OPS, _SUB_OPCODE_FOR_NAME
    from concourse.dve_spec import (
        C0,
        C1,
        One,
        Spec,
        Src0,
        Src1,
        _has_src1,
        lower,
        maxx,
        relu,
    )
    from concourse.dve_uop import DveOpSpec

    existing = {op.name: op for op in OPS}

    def mk(key, name, body):
        if name in existing:
            _OPS_CACHE[key] = existing[name]
            return
        if name not in _SUB_OPCODE_FOR_NAME:
            row = max(_SUB_OPCODE_FOR_NAME.values()) + 1
            assert row < 0x20, "custom DVE opcode rows exhausted"
            _SUB_OPCODE_FOR_NAME[name] = row
        spec = Spec(body=body)
        shas = {}
        for ver in ("v3", "v4"):
            uops = lower(spec, ver=ver)
            s = DveOpSpec(
                name=name,
                opcode=_SUB_OPCODE_FOR_NAME[name],
                uops=uops,
                rd1_en=_has_src1(spec),
            )
            shas[ver] = s.sha(ver)
        op = DveOp(name, spec, subdim=False, uops_sha=shas)
        OPS.append(op)
        dve_ops.CUSTOM_DVE_SPECS[name] = spec
        _OPS_CACHE[key] = op

    # tau candidates: max((sm - mn - 1)*0.5, (sm - 1)/3)
    mk("tau_a", "ANT_TAU_A", maxx((Src0 - Src1 - One) * C0, (Src0 - One) * C1))
    # tau = max(mx - 1, d)
    mk("tau_b", "ANT_TAU_B", maxx(Src0 - One, Src1))
    # sparsemax threshold: relu(x - tau)
    mk("relusub", "ANT_RELUSUB", relu(Src0 - Src1))
    # z = max(a + b, eps)
    mk("addmax", "ANT_ADDMAX", maxx(Src0 + Src1, C0))
    # cos = a*b + 0.1
    mk("fmac", "ANT_FMA_C", Src0 * Src1 + C0)
    # scale = (a*21)*b
    mk("smul", "ANT_SMUL", (Src0 * C0) * Src1)
    # out = max(a*b, 0.001)
    mk("maxmul", "ANT_MAXMUL", maxx(Src0 * Src1, C0))
    return _OPS_CACHE


# --------------------------------------------------------------------------
# Bass program
# --------------------------------------------------------------------------
_PROG_CACHE: dict = {}


def _build_program(z_eps: float, scale_factor: float, B: int, T: int):
    """One SPMD program; every core runs the same code on its own shard."""
    ops = _custom_ops()
    # Bacc (not raw Bass): its compile() runs generate_event_semaphores,
    # which legalizes multi-event-sem waits the DVE/CTRL structs can't carry.
    nc = bacc.Bacc(
        "TRN2",
        target_bir_lowering=False,
        num_devices=N_CORES,
        dynamic_dma_scratch_size=8192,
    )
    e_pad = PDIM * B * T

    # Single fused input stream per tile (one DMA -> one DMA-sem wait per
    # consumer; the DVE TT struct only tolerates a single event-sem wait).
    # Per tile, per partition: [3B child | 3B parent | 12B gathered-table].
    xin_d = nc.dram_tensor("xin", [e_pad * 18], F32, kind="ExternalInput")
    out_d = nc.dram_tensor("alpha", [e_pad * 3], F32, kind="ExternalOutput")

    xin_v = xin_d[:].rearrange("(t p c) -> t p c", t=T, p=PDIM)
    out_v = out_d[:].rearrange("(t p c) -> t p c", t=T, p=PDIM)

    with TileContext(nc) as tc:
        with (
            tc.tile_pool(name="io", bufs=2) as iop,
            tc.tile_pool(name="scr", bufs=1) as sp,
        ):
            for t in range(T):
                xin = iop.tile([PDIM, 18 * B], F32, tag="xin", name=f"xin{t}")
                ot = iop.tile([PDIM, 3 * B], F32, tag="ot", name=f"ot{t}")
                nc.sync.dma_start(xin[:], xin_v[t])

                # Planar layout: 18 contiguous planes of B per partition:
                # [cp0 cp1 cp2 | pp0 pp1 pp2 | m00..m22 | b0 b1 b2]
                cp_all = xin[:, 0 : 3 * B]
                pp_all = xin[:, 3 * B : 6 * B]
                mrow = [xin[:, (6 + 3 * i) * B : (9 + 3 * i) * B] for i in range(3)]
                b_all = xin[:, 15 * B : 18 * B]

                # output stays planar (3 planes of B); host re-interleaves

                def pl(tag):
                    return sp.tile(
                        [PDIM, B], F32, tag=tag, name=f"{tag}{t}", bufs=2
                    )[:]

                V = nc.vector
                A = nc.scalar

                def cdve(op_key, out, in0, in1, s0=0.0, s1=0.0, imm2=0.0):
                    V._custom_dve(
                        ops[op_key], out=out, in0=in0, in1=in1, s0=s0, s1=s1, imm2=imm2
                    )

                # ww: 6 planes; first half doubles as w1, second as l_all.
                # Keeping them in ONE tile lets paired sum-chains run as
                # single strided-wide adds into separate 2-plane outputs.
                ww = sp.tile([PDIM, 6 * B], F32, tag="ww", name=f"ww{t}", bufs=2)[:]
                w1 = ww[:, 0 : 3 * B]
                l_all = ww[:, 3 * B : 6 * B]
                c_all = sp.tile([PDIM, 3 * B], F32, tag="call", name=f"call{t}")[:]
                p_all = sp.tile([PDIM, 3 * B], F32, tag="pall", name=f"pall{t}")[:]

                def bcast3(plane_ap):
                    """[P,B] plane broadcast to [P,3,B] via a 0-stride dim."""
                    return plane_ap.rearrange("p (u b) -> p u b", u=1).broadcast_to(
                        [PDIM, 3, B]
                    )

                def pair_sum3(src6, out2):
                    """out2[:, k*B+e] = sum_j src6[:, (3k+j)*B+e] for k=0,1."""
                    v = src6.rearrange("p (n b) -> p n b", b=B)
                    x = v[:, 0:6:3, :]
                    y = v[:, 1:6:3, :]
                    z = v[:, 2:6:3, :]
                    o = out2.rearrange("p (n b) -> p n b", b=B)
                    V.tensor_add(o, x, y)
                    V.tensor_add(o, o, z)

                # ---- c_raw = M[rel] @ child: rows 0,1 in one 6-plane mul
                # (cp broadcast across the two rows), row 2 separately.
                cp2 = cp_all.rearrange("p (u x) -> p u x", u=1).broadcast_to(
                    [PDIM, 2, 3 * B]
                )
                V.tensor_mul(
                    ww.rearrange("p (u x) -> p u x", x=3 * B),
                    xin[:, 6 * B : 12 * B].rearrange("p (u x) -> p u x", x=3 * B),
                    cp2,
                )
                pair_sum3(ww, c_all[:, 0 : 2 * B])  # c0|c1
                V.tensor_mul(w1, mrow[2], cp_all)
                c2 = c_all[:, 2 * B : 3 * B]
                V.tensor_add(c2, w1[:, 0:B], w1[:, B : 2 * B])
                V.tensor_add(c2, c2, w1[:, 2 * B : 3 * B])

                # ---- sparsemax (simplex projection, d=3):
                # tau = max(mx-1, (sm-mn-1)/2, (sm-1)/3); out = relu(x - tau)
                def sparsemax(x_all, out_all, tag):
                    x = [x_all[:, j * B : (j + 1) * B] for j in range(3)]
                    mx, mn, sm, dd = (
                        pl(f"mx{tag}"),
                        pl(f"mn{tag}"),
                        pl(f"sm{tag}"),
                        pl(f"dd{tag}"),
                    )
                    V.tensor_max(mx, x[0], x[1])
                    V.tensor_max(mx, mx, x[2])
                    V.tensor_tensor(mn, x[0], x[1], OP.min)
                    V.tensor_tensor(mn, mn, x[2], OP.min)
                    V.tensor_add(sm, x[0], x[1])
                    V.tensor_add(sm, sm, x[2])
                    cdve("tau_a", dd, sm, mn, s0=0.5, s1=1.0 / 3.0)
                    cdve("tau_b", dd, mx, dd)
                    ov = out_all.rearrange("p (n b) -> p n b", b=B)
                    xv = x_all.rearrange("p (n b) -> p n b", b=B)
                    cdve("relusub", ov, xv, bcast3(dd))
                    return mx, mn, sm, dd

                # c = sparsemax(c_raw); the reference's second application is
                # an exact no-op (projection idempotence) up to ~1e-7: skipped.
                lzs = sparsemax(c_all, c_all, "c")[0]  # mx plane recycled
                ncs = sparsemax(pp_all, p_all, "p")[0]  # mx plane recycled

                # ---- z = max(p + c, eps); entropy = ln(zs) - sum(z ln z)/zs
                # zq = [zs | szl] computed as one paired sum over ww
                w1v = w1.rearrange("p (n b) -> p n b", b=B)
                cvv = c_all.rearrange("p (n b) -> p n b", b=B)
                pvv = p_all.rearrange("p (n b) -> p n b", b=B)
                cdve("addmax", w1v, cvv, pvv, s0=z_eps)  # w1 <- z (wide)
                A.activation(l_all, w1, AF.Ln)  # wide ln
                V.tensor_mul(l_all, w1, l_all)  # l <- z*ln z (wide)
                zq = sp.tile([PDIM, 2 * B], F32, tag="zq", name=f"zq{t}", bufs=2)[:]
                pair_sum3(ww, zq)  # zq <- [zs | szl]
                zs = zq[:, 0:B]
                szl = zq[:, B : 2 * B]
                A.activation(lzs, zs, AF.Ln)
                A.activation(zs, lzs, AF.Exp, scale=-1.0)  # zs <- 1/zs
                V.tensor_mul(szl, szl, zs)
                V.tensor_sub(lzs, lzs, szl)  # lzs <- entropy

                # ---- cos = 0.1 + (p.c) / sqrt((p.p)*(c.c))
                V.tensor_mul(w1, p_all, c_all)  # w1 <- pc (wide)
                A.square(l_all, p_all)  # wide
                nq = sp.tile([PDIM, 2 * B], F32, tag="nq", name=f"nq{t}", bufs=2)[:]
                pair_sum3(ww, nq)  # nq <- [p.c | p.p]
                num = nq[:, 0:B]
                nps = nq[:, B : 2 * B]
                A.square(l_all, c_all)  # wide
                V.tensor_add(ncs, l_all[:, 0:B], l_all[:, B : 2 * B])
                V.tensor_add(ncs, ncs, l_all[:, 2 * B : 3 * B])
                # 1/(|p||c|) = exp(-(ln nps + ln ncs)/2)
                A.activation(nps, nps, AF.Ln)
                A.activation(ncs, ncs, AF.Ln)
                V.tensor_add(ncs, nps, ncs)
                A.activation(ncs, ncs, AF.Exp, scale=-0.5)
                cdve("fmac", num, num, ncs, s0=0.1)  # num <- cos

                # ---- alpha0 = p + b*(c-p) (wide, into w1)
                V.tensor_sub(w1, c_all, p_all)
                V.tensor_mul(w1, w1, b_all)
                V.tensor_add(w1, p_all, w1)

                # ---- scale = sf*cos/entropy; out = max(alpha0*scale, 1e-3)
                # 1/entropy = exp(-ln entropy)
                A.activation(lzs, lzs, AF.Ln)
                A.activation(lzs, lzs, AF.Exp, scale=-1.0)
                cdve("smul", num, num, lzs, s0=scale_factor)  # num <- scale
                cdve("maxmul", ot[:].rearrange("p (n b) -> p n b", b=B), w1v,
                     bcast3(num), s0=0.001)

                nc.sync.dma_start(out_v[t], ot[:])

    nc.compile()
    return nc


def _get_program(z_eps: float, scale_factor: float, B: int, T: int):
    key = (round(z_eps, 9), round(scale_factor, 9), B, T)
    if key not in _PROG_CACHE:
        _PROG_CACHE[key] = _build_program(z_eps, scale_factor, B, T)
    return _PROG_CACHE[key]


# --------------------------------------------------------------------------
# Host entry point
# --------------------------------------------------------------------------
_B = 656
_T = 12
E_PAD = PDIM * _B * _T  # 1,007,616 >= 1,000,000


def _fused_shard(
    child: np.ndarray,
    prnt: np.ndarray,
    g: np.ndarray,
    k: int,
    e: int,
    e_pad: int,
    B: int,
    T: int,
) -> np.ndarray:
    """Per-core fused input stream [T, 128, 18, B]: per tile & partition, 18
    contiguous planes of B: [cp0 cp1 cp2 | pp0 pp1 pp2 | m00..m22 | b0 b1 b2],
    flattened f32 (planar keeps every device access contiguous)."""

    def pad(a):
        sl = a[k * e : (k + 1) * e]
        out = np.zeros((e_pad, a.shape[1]), dtype=np.float32)
        out[: sl.shape[0]] = sl
        # [e_pad, w] -> [T, 128, B, w] -> planar [T, 128, w, B]
        return out.reshape(T, PDIM, B, -1).transpose(0, 1, 3, 2)

    return np.ascontiguousarray(
        np.concatenate([pad(child), pad(prnt), pad(g)], axis=2)
    ).reshape(-1)


def _run(inputs: dict, trace: bool = False):
    child = np.asarray(inputs["child_probs"], dtype=np.float32)
    prnt = np.asarray(inputs["prnt_probs"], dtype=np.float32)
    M = np.asarray(inputs["M"], dtype=np.float32)
    beta = np.asarray(inputs["beta"], dtype=np.float32)
    rels = np.asarray(inputs["rels"])
    z_eps = float(np.asarray(inputs["z_epsilon"]))
    sf = float(np.asarray(inputs["scale_factor"]))

    n = rels.shape[0]
    assert n % N_CORES == 0
    e = n // N_CORES
    assert e <= E_PAD

    t12 = np.concatenate([M.reshape(M.shape[0], 9), beta], axis=1).astype(np.float32)
    g = t12[rels]  # [N, 12]

    nc = _get_program(z_eps, sf, _B, _T)
    in_maps = [
        {"xin": _fused_shard(child, prnt, g, k, e, E_PAD, _B, _T)}
        for k in range(N_CORES)
    ]
    res = run_bass_kernel_spmd(nc, in_maps, core_ids=list(range(N_CORES)), trace=trace)
    # device emits planar [T, 128, 3, B]; re-interleave to [E, 3]
    outs = [
        r["alpha"]
        .reshape(_T, PDIM, 3, _B)
        .transpose(0, 1, 3, 2)
        .reshape(E_PAD, 3)[:e]
        for r in res.results
    ]
    return np.concatenate(outs, axis=0), res


def kernel(**inputs) -> np.ndarray:
    out, _ = _run(inputs)
    return out


def kernel_traced(**inputs):
    """Returns (output, BassKernelResults-with-profile) for test harnesses."""
    return _run(inputs, trace=True)
